# revision 19
# baseline (speedup 1.0000x reference)
"""Trainium2 Bass kernel for the nn_AaD retrieval-KNN loss.

Self-contained: takes the FULL unsharded inputs, shards fea_bank/score_bank
row-wise across 8 NeuronCores, runs one SPMD Bass program per core:
  - bf16 distance matmul (f_normT stationary) over the core's 6250-row slab,
  - unit-split hardware top-8 (MAX8/FIND_INDEX8) pipelined under the load,
  - candidate merge + one full-row FIND_INDEX8 for absolute top-6 indices,
  - indirect-DMA gather of candidate score rows (bf16),
  - per-candidate KL contribution g = sum sb*(ln sb - p) via fused STT,
  - dispersion via the rank-1 identity (||sum p||^2 - sum ||p||^2)/B.
Host merges the per-core candidates (exact fp32 re-ranking) into the loss.
"""

import numpy as np
import ml_dtypes

import concourse.mybir as mybir
import concourse.tile as tile
from concourse import bacc
from concourse.bass import IndirectOffsetOnAxis
from concourse.bass_utils import run_bass_kernel_spmd

B, D, C, N, K = 256, 512, 345, 50000, 5
ALPHA = 1.0
EPS = 1e-12
M = 8                   # cores
NS = N // M             # 6250 bank rows per core
G = 13                  # 512-wide column groups per core
GW = 512
NPAD = G * GW           # 6656
LASTW = NS - (G - 1) * GW
JJ = 6                  # candidate slots kept per core (global top-6 needs 6)

# group partition into DMA chunks == top-k units
UNIT_GROUPS = [4, 4, 4, 1]
UNIT_STARTS_G = [0, 4, 8, 12]
NU = len(UNIT_GROUPS)

F32 = mybir.dt.float32
BF16 = mybir.dt.bfloat16
U32 = mybir.dt.uint32
AF = mybir.ActivationFunctionType
ALU = mybir.AluOpType

_CACHE: dict = {}


def _build():
    nc = bacc.Bacc("TRN2", target_bir_lowering=False, debug=False, num_devices=M)

    # per-partition contiguous layout: fbt[p, g*2048 + dk*512 + c]
    fbt_in = nc.dram_tensor("fbt", [128, G * 4 * GW], BF16, kind="ExternalInput")
    fnt_in = nc.dram_tensor("fnt", [128, 4 * B], BF16, kind="ExternalInput")
    sbk_in = nc.dram_tensor("sbk", [NS, C], F32, kind="ExternalInput")
    p_in = nc.dram_tensor("p", [128, 2 * C], BF16, kind="ExternalInput")

    out_vals = nc.dram_tensor("out_vals", [2, 128, 8], F32, kind="ExternalOutput")
    out_g = nc.dram_tensor("out_g", [2, 128, JJ], F32, kind="ExternalOutput")
    out_idx = nc.dram_tensor("out_idx", [2, 128, JJ], U32, kind="ExternalOutput")
    out_disp = nc.dram_tensor("out_disp", [1, 1], F32, kind="ExternalOutput")
    junk_out = nc.dram_tensor("junk_out", [1, 8], F32, kind="ExternalOutput")

    with tile.TileContext(nc) as tc:
        with (
            tc.tile_pool(name="const", bufs=1) as constp,
            tc.tile_pool(name="small", bufs=2) as smallp,
            tc.tile_pool(name="scr", bufs=2) as scrp,
            tc.tile_pool(name="psum", bufs=5, space="PSUM") as psp,
            tc.tile_pool(name="psum2", bufs=1, space="PSUM") as psp2,
        ):
            fnt_sb = constp.tile([128, 4 * B], BF16, tag="fnt")
            nc.sync.dma_start(fnt_sb[:], fnt_in[:])
            p_sb = constp.tile([128, 2 * C], BF16, tag="psb")
            nc.sync.dma_start(p_sb[:], p_in[:])

            # PE warm-up: dummy matmuls (junk values) keep TensorE busy during
            # the fbt load so HAM promotes the clock before the real matmuls.
            junk_ps = psp2.tile([128, GW], F32, tag="junk", space="PSUM")
            for wi in range(8):
                nc.tensor.matmul(junk_ps[:], lhsT=fnt_sb[:, 0:128],
                                 rhs=fnt_sb[:, 0:GW], start=(wi == 0), stop=(wi == 7))
            junk_sb = constp.tile([1, 8], F32, tag="junksb")
            nc.scalar.activation(junk_sb[:], junk_ps[:1, :8], AF.Copy)
            nc.sync.dma_start(junk_out[:], junk_sb[:])

            # whole fbt slab persistent in SBUF; loaded in unit-aligned chunks
            fbt_sb = constp.tile([128, G * 4 * GW], BF16, tag="fbt")
            dists = [constp.tile([128, NS], F32, tag=f"dist{m}", name=f"dist{m}")
                     for m in range(2)]

            cand_vals = [constp.tile([128, NU * 8], F32, tag=f"cv{m}", name=f"cv{m}")
                         for m in range(2)]
            dma_spans = [(0, 2), (2, 4), (4, 7), (7, 10), (10, 13)]
            for (ga, gb) in dma_spans:
                nc.sync.dma_start(fbt_sb[:, ga * 4 * GW:gb * 4 * GW],
                                  fbt_in[:, ga * 4 * GW:gb * 4 * GW])
            def emit_group(g, m):
                w = GW if g < G - 1 else LASTW
                ps = psp.tile([128, GW], F32, tag="ps", name=f"ps{m}_{g % 3}")
                for dk in range(4):
                    nc.tensor.matmul(
                        ps[:],
                        lhsT=fnt_sb[:, dk * B + m * 128: dk * B + m * 128 + 128],
                        rhs=fbt_sb[:, g * 4 * GW + dk * GW: g * 4 * GW + (dk + 1) * GW],
                        start=(dk == 0),
                        stop=(dk == 3),
                    )
                nc.scalar.activation(dists[m][:, g * GW:g * GW + w], ps[:, :w], AF.Copy)

            LASTCHUNK = 10
            for g in range(LASTCHUNK):
                for m in range(2):
                    emit_group(g, m)
            # last chunk: finish batch-tile 0 entirely first so its top-k/FIND
            # overlaps batch-tile 1's matmul tail
            for g in range(LASTCHUNK, G):
                emit_group(g, 0)
            for g in range(LASTCHUNK, G):
                emit_group(g, 1)

            # merge per batch tile: top-8 of the 32 candidates, then select
            # the top-JJ original indices via is_equal/mult STT against iota.
            for m in range(2):
                for u in range(NU):
                    g0 = UNIT_STARTS_G[u]
                    d0 = g0 * GW
                    d1 = min((g0 + UNIT_GROUPS[u]) * GW, NS)
                    uvs = cand_vals[m][:, u * 8:(u + 1) * 8]
                    nc.vector.max(out=uvs, in_=dists[m][:, d0:d1])

                mx8 = smallp.tile([128, 8], F32, tag=f"mx8_{m}")
                nc.vector.max(out=mx8[:], in_=cand_vals[m][:])
                # absolute positions of the merged top-8 in one full-row pass
                sel8 = smallp.tile([128, 8], U32, tag=f"sel8_{m}")
                nc.vector.max_index(out=sel8[:], in_max=mx8[:], in_values=dists[m][:])
                sel_u = sel8[:, 0:JJ]
                nc.sync.dma_start(out_vals[m], mx8[:])
                nc.sync.dma_start(out_idx[m], sel8[:, 0:JJ])

                # gather candidate score rows (bf16), per-slot tiles for fine deps
                hq = smallp.tile([128, 2 * JJ], F32, tag=f"hq{m}")
                for j in range(JJ):
                    sbg = smallp.tile([128, C], F32, tag=f"sbg{j % 3}", name=f"sbg{m}_{j % 3}")
                    nc.gpsimd.indirect_dma_start(
                        out=sbg[:],
                        out_offset=None,
                        in_=sbk_in[:, :],
                        in_offset=IndirectOffsetOnAxis(ap=sel8[:, j:j + 1], axis=0),
                    )
                    # q_j = sum_c sb*p  (no ln dependency)
                    scrq = scrp.tile([128, C], F32, tag=f"scrq{j % 2}", name=f"scrq{m}_{j % 2}")
                    nc.vector.scalar_tensor_tensor(
                        out=scrq[:], in0=p_sb[:, m * C:(m + 1) * C], scalar=0.0,
                        in1=sbg[:], op0=ALU.add, op1=ALU.mult,
                        accum_out=hq[:, JJ + j:JJ + j + 1],
                    )
                    tln = smallp.tile([128, C], F32, tag=f"tln{j % 3}", name=f"tln{m}_{j % 3}")
                    nc.scalar.activation(tln[:], sbg[:], AF.Ln)
                    scrh = scrp.tile([128, C], F32, tag=f"scrh{j % 2}", name=f"scrh{m}_{j % 2}")
                    nc.vector.scalar_tensor_tensor(
                        out=scrh[:], in0=tln[:], scalar=0.0,
                        in1=sbg[:], op0=ALU.add, op1=ALU.mult,
                        accum_out=hq[:, j:j + 1],
                    )
                g8 = smallp.tile([128, JJ], F32, tag=f"g8{m}")
                nc.vector.tensor_tensor(out=g8[:], in0=hq[:, 0:JJ], in1=hq[:, JJ:2 * JJ], op=ALU.subtract)
                nc.sync.dma_start(out_g[m], g8[:])

            # dispersion: (||sum_b p_b||^2 - sum_b ||p_b||^2) / B
            ones = constp.tile([128, 1], BF16, tag="ones")
            nc.vector.memset(ones[:], 1.0)
            rowsq = constp.tile([128, 2], F32, tag="rowsq")
            for m in range(2):
                scr = scrp.tile([128, C], BF16, tag="scrd", name=f"scrd{m}")
                nc.vector.scalar_tensor_tensor(
                    out=scr[:], in0=p_sb[:, m * C:(m + 1) * C], scalar=0.0,
                    in1=p_sb[:, m * C:(m + 1) * C], op0=ALU.add, op1=ALU.mult,
                    accum_out=rowsq[:, m:m + 1],
                )
            rowsq_b = constp.tile([128, 2], BF16, tag="rowsqb")
            nc.vector.tensor_copy(rowsq_b[:], rowsq[:])
            s_ps = psp2.tile([1, C], F32, tag="S")
            for m in range(2):
                nc.tensor.matmul(
                    s_ps[:], lhsT=ones[:], rhs=p_sb[:, m * C:(m + 1) * C],
                    start=(m == 0), stop=(m == 1),
                )
            rq_ps = psp2.tile([1, 2], F32, tag="rq")
            nc.tensor.matmul(rq_ps[:], lhsT=ones[:], rhs=rowsq_b[:], start=True, stop=True)

            s_sb = constp.tile([1, C], F32, tag="ssb")
            nc.vector.tensor_copy(s_sb[:], s_ps[:])
            rq_sb = constp.tile([1, 2], F32, tag="rqsb")
            nc.vector.tensor_copy(rq_sb[:], rq_ps[:])
            scr_s = constp.tile([1, C], F32, tag="scrS")
            ssq = constp.tile([1, 1], F32, tag="ssq")
            nc.vector.scalar_tensor_tensor(
                out=scr_s[:], in0=s_sb[:], scalar=0.0, in1=s_sb[:],
                op0=ALU.add, op1=ALU.mult, accum_out=ssq[:],
            )
            t1 = constp.tile([1, 1], F32, tag="t1")
            nc.vector.tensor_tensor(out=t1[:], in0=rq_sb[:, 0:1], in1=rq_sb[:, 1:2], op=ALU.add)
            t2 = constp.tile([1, 1], F32, tag="t2")
            nc.vector.tensor_tensor(out=t2[:], in0=ssq[:], in1=t1[:], op=ALU.subtract)
            t3 = constp.tile([1, 1], F32, tag="t3")
            nc.vector.tensor_scalar_mul(t3[:], t2[:], 1.0 / B)
            nc.sync.dma_start(out_disp[:], t3[:])

    nc.compile()
    return nc


def _get_nc():
    if "nc" not in _CACHE:
        _CACHE["nc"] = _build()
    return _CACHE["nc"]


def _prep(features, predictions, fea_bank, score_bank, trg_idx):
    feat = np.asarray(features, dtype=np.float32)
    pred = np.asarray(predictions, dtype=np.float32)
    fb = np.array(fea_bank, dtype=np.float32)
    sb = np.array(score_bank, dtype=np.float32)
    trg = np.asarray(trg_idx).astype(np.int64)

    x = pred - pred.max(axis=1, keepdims=True)
    e = np.exp(x)
    p = e / e.sum(axis=1, keepdims=True)

    nrm = np.sqrt((feat * feat).sum(axis=1, keepdims=True))
    fn = feat / np.maximum(nrm, EPS)

    fb[trg] = fn
    sb[trg] = p

    fnt = np.ascontiguousarray(fn.T.reshape(4, 128, B).transpose(1, 0, 2)
                               .reshape(128, 4 * B).astype(ml_dtypes.bfloat16))
    p_dev = np.ascontiguousarray(p.reshape(2, 128, C).transpose(1, 0, 2)
                                 .reshape(128, 2 * C).astype(ml_dtypes.bfloat16))

    in_maps = []
    for c in range(M):
        slab = fb[c * NS:(c + 1) * NS]
        fbt = np.zeros((D, NPAD), dtype=np.float32)
        fbt[:, :NS] = slab.T
        # [dk, p, g, c] -> per-partition contiguous [p, (g dk c)]
        fbt = np.ascontiguousarray(
            fbt.reshape(4, 128, G, GW).transpose(1, 2, 0, 3).reshape(128, G * 4 * GW)
            .astype(ml_dtypes.bfloat16))
        sbk = np.ascontiguousarray(sb[c * NS:(c + 1) * NS])
        in_maps.append({"fbt": fbt, "fnt": fnt, "sbk": sbk, "p": p_dev})
    return in_maps, fn, fb


def _merge(results, fn, fb):
    gs, gidx = [], []
    for c in range(M):
        r = results[c]
        gs.append(r["out_g"].reshape(B, JJ))
        gidx.append(r["out_idx"].reshape(B, JJ).astype(np.int64) + c * NS)
    g = np.concatenate(gs, axis=1)
    gi = np.concatenate(gidx, axis=1)

    # exact fp32 re-ranking of the candidates (kills bf16 rank perturbation)
    cand_rows = fb[gi]                                   # [B, M*JJ, D]
    v = np.einsum("bd,bkd->bk", fn, cand_rows).astype(np.float32)

    # global top-(K+1) by value, ties -> lowest original index (lax.top_k)
    order = np.lexsort((gi, -v.astype(np.float64)), axis=-1)
    sel = order[:, 1:K + 1]  # drop rank 0
    kl = np.take_along_axis(g, sel, axis=1).astype(np.float64).sum(axis=1).mean()
    disp = float(results[0]["out_disp"][0, 0])
    return np.float32(kl + ALPHA * disp)


def run(inputs, trace=False):
    nc = _get_nc()
    in_maps, fn, fb = _prep(**inputs)
    res = run_bass_kernel_spmd(nc, in_maps, list(range(M)), trace=trace)
    return _merge(res.results, fn, fb), res


def kernel(features, predictions, fea_bank, score_bank, trg_idx):
    loss, _ = run(
        dict(
            features=features,
            predictions=predictions,
            fea_bank=fea_bank,
            score_bank=score_bank,
            trg_idx=trg_idx,
        )
    )
    return loss
